# revision 1
# baseline (speedup 1.0000x reference)
"""Trainium2 Bass kernel for nn_DecoderBlock_Mamba (AxialDW conv + 1x1 conv +
BN + ReLU + LN + Mamba selective scan + residual).

Sharding: 8 cores = (batch b in 0..3) x (state-half sigma in {0,1}).
Each core runs the full per-image pipeline for its batch element but only 8 of
the 16 SSM states; partial y is AllReduce'd within core pairs, post-stack is
computed redundantly on both cores of a pair.

Self-contained: hardcodes all shapes; no sibling imports.
"""
import numpy as np

C = 64
DI = 128
DS = 16
DR = 4
B = 4
H = 64
W = 64
L = H * W
NS = 8            # states per core
NCORES = 8
ROW = W + 2       # padded row stride
LP = (H + 2) * ROW
NCH = 8           # L chunks of 512
CH = 512
EPS = 1e-5

_cached = {}


def _build_program(sim=False, phases=3):
    import concourse.bass as bass
    import concourse.bacc as bacc
    import concourse.mybir as mybir
    import concourse.tile as tile

    dt = mybir.dt
    f32 = dt.float32
    bf16 = dt.bfloat16
    Act = mybir.ActivationFunctionType
    Alu = mybir.AluOpType
    Axis = mybir.AxisListType

    nc = bacc.Bacc(None, target_bir_lowering=False)

    def din(name, shape, dtype=f32):
        return nc.dram_tensor(name, shape, dtype, kind="ExternalInput")

    ximgs_d = din("ximgs", [C, 5 * L], bf16)
    cf32_d = din("cf32", [128, 19])
    cbf_d = din("cbf", [128, 2948], bf16)

    out_d = nc.dram_tensor("out_f", [C, L], f32, kind="ExternalOutput")

    groups = [[0, 1], [2, 3], [4, 5], [6, 7]]

    with tile.TileContext(nc) as tc:
        with (
            tc.tile_pool(name="dram", bufs=1, space="DRAM") as dpool,
            tc.tile_pool(name="const", bufs=1) as cpool,
            tc.tile_pool(name="big", bufs=1) as bpool,
            tc.tile_pool(name="sm", bufs=2) as spool,
            tc.tile_pool(name="da", bufs=2) as dapool,
            tc.tile_pool(name="dbx", bufs=2) as dbxpool,
            tc.tile_pool(name="ps", bufs=4, space="PSUM") as ps,
            tc.tile_pool(name="psy", bufs=2, space="PSUM") as psy,
        ):
            # ---- load constants (packed: 3 DMAs total) ----
            cf = cpool.tile([128, 19], f32)
            cb = cpool.tile([128, 2948], bf16)
            nc.sync.dma_start(cf[:], cf32_d[:])
            nc.sync.dma_start(cb[:], cbf_d[:])
            bn_s = cf[0:C, 0:1]
            bn_b = cf[0:C, 1:2]
            ip_b = cf[:, 2:4]
            cd_w = cf[:, 4:8]
            cd_b = cf[:, 8:9]
            dt_b = cf[:, 9:10]
            a_sc = cf[:, 10:18]
            Dp = cf[:, 18:19]
            ident = cb[:, 0:128]
            cw = cb[0:C, 128:448]
            ip_lhsT = cb[0:C, 448:704]
            xpdt_lhsT = cb[:, 704:708]
            dt_lhsT = cb[0:DR, 708:836]
            brep_lhsT = cb[:, 836:1860]
            crep_lhsT = cb[:, 1860:2884]
            op_lhsT = cb[:, 2884:2948]

            # ---- persistent activations ----
            SEQ = bpool.tile([C, L], bf16)           # BN+ReLU output (residual)
            HN = bpool.tile([C, L], bf16)            # LN-normalized (no affine)
            XM0 = bpool.tile([DI, L + 4], bf16)      # conv1d input, data @ col 4
            ZS = bpool.tile([DI, L], bf16)           # silu(z)
            XC = bpool.tile([DI, L], bf16)
            DT = bpool.tile([DI, L], bf16)
            U = bpool.tile([DI, L], bf16)
            Hs = [bpool.tile([DI, L], bf16, name=f"H{j}", tag=f"H{j}") for j in range(NS)]
            YSUM = bpool.tile([DI, L], bf16, name="YSUM", tag="U")

            # Prime ACT's vector clock on the const DMAs so later
            # activations (limited wait slots) don't re-wait on them.
            warm = cpool.tile([128, 1], f32, tag="warm")
            nc.scalar.activation(warm[:], cf[:, 0:1], Act.Copy)
            warm2 = cpool.tile([128, 1], bf16, tag="warm2")
            nc.scalar.activation(warm2[:], cb[:, 0:1], Act.Copy)
            eps_t = cpool.tile([128, 1], f32, tag="epsl")
            nc.gpsimd.memset(eps_t[:], EPS)
            nc.vector.tensor_scalar_mul(XM0[:, 0:4], cf[:, 0:4], 0.0)

            IMGS = [bpool.tile([C, L], bf16, name=f"img{t}", tag=f"H{t}")
                    for t in range(5)]
            for t in range(5):
                nc.sync.dma_start(IMGS[t][:], ximgs_d[:, t * L:(t + 1) * L])

            # ---- front conv: 5 accumulating taps + BN + ReLU ----
            for chi in range(NCH):
                sl = slice(chi * CH, (chi + 1) * CH)
                pc = ps.tile([C, CH], f32, tag="mm")
                for tap in range(5):
                    nc.tensor.matmul(pc[:], cw[:, tap * C:(tap + 1) * C],
                                     IMGS[tap][:, sl],
                                     start=(tap == 0), stop=(tap == 4))
                nc.scalar.activation(SEQ[:, chi * CH:(chi + 1) * CH], pc[:],
                                     Act.Relu, bias=bn_b, scale=bn_s)

            # ---- LayerNorm over channels, batched 4 blocks per DVE op ----
            HN0 = bpool.tile([128, L // 2], bf16, name="HN0", tag="HN0")
            VARS = spool.tile([128, 32], f32, tag="VARS")
            NG = L // 512  # 8 groups of 4 128-token blocks
            for g in range(NG if phases >= 1 else 0):
                tps4 = ps.tile([128, 4, C], bf16, tag="mm")
                for k in range(4):
                    blk = g * 4 + k
                    nc.tensor.transpose(tps4[:, k, :],
                                        SEQ[:, blk * 128:(blk + 1) * 128],
                                        ident[0:C, 0:C])
                mu4 = spool.tile([128, 4], f32, tag="mu4")
                nc.vector.tensor_reduce(mu4[:], tps4[:], Axis.X, Alu.add)
                mun4 = spool.tile([128, 4], f32, tag="mun4")
                nc.vector.tensor_scalar_mul(mun4[:], mu4[:], 1.0 / C)
                h04 = HN0[:, g * 256:(g + 1) * 256].rearrange(
                    "p (b c) -> p b c", b=4)
                nc.vector.tensor_tensor(h04, tps4[:],
                                        mun4[:].to_broadcast((128, 4, C)),
                                        op=Alu.subtract)
                sq4 = spool.tile([128, 4, C], f32, tag="sq4")
                nc.vector.tensor_mul(sq4[:], h04, h04)
                ssq4 = spool.tile([128, 4], f32, tag="ssq4")
                nc.vector.tensor_reduce(ssq4[:], sq4[:], Axis.X, Alu.add)
                nc.vector.tensor_scalar(VARS[:, g * 4:(g + 1) * 4], ssq4[:],
                                        1.0 / C, EPS,
                                        op0=Alu.mult, op1=Alu.add)
            SQV = spool.tile([128, 32], f32, tag="SQV")
            RSTD = spool.tile([128, 32], f32, tag="RSTD")
            if phases >= 1:
                nc.scalar.activation(SQV[:], VARS[:], Act.Sqrt)
                nc.vector.reciprocal(RSTD[:], SQV[:])
            HNT = bpool.tile([128, L // 2], bf16, name="HNT", tag="HNT")
            for g in range(NG if phases >= 1 else 0):
                hnT4 = HNT[:, g * 256:(g + 1) * 256].rearrange(
                    "p (b c) -> p b c", b=4)
                nc.vector.tensor_tensor(
                    hnT4, HN0[:, g * 256:(g + 1) * 256].rearrange(
                        "p (b c) -> p b c", b=4),
                    RSTD[:, g * 4:(g + 1) * 4].to_broadcast((128, 4, C)),
                    op=Alu.mult)
                tb4 = ps.tile([C, 4, 128], bf16, tag="mm")
                for k in range(4):
                    blk = g * 4 + k
                    nc.tensor.transpose(tb4[:, k, :],
                                        HNT[:, blk * C:(blk + 1) * C],
                                        ident)
                nc.scalar.activation(HN[:, g * CH:(g + 1) * CH],
                                     tb4[:].rearrange("p a b -> p (a b)"),
                                     Act.Copy)
            # ---- in_proj ----
            for chi in range(NCH if phases >= 1.5 else 0):
                sl = slice(chi * CH, (chi + 1) * CH)
                xm_ps = ps.tile([DI, CH], f32, tag="mm")
                z_ps = ps.tile([DI, CH], f32, tag="mm")
                nc.tensor.matmul(xm_ps[:], ip_lhsT[0:C, 0:DI], HN[:, sl],
                                 start=True, stop=True)
                nc.tensor.matmul(z_ps[:], ip_lhsT[0:C, DI:2 * DI], HN[:, sl],
                                 start=True, stop=True)
                nc.scalar.activation(XM0[:, 4 + chi * CH:4 + (chi + 1) * CH],
                                     xm_ps[:], Act.Identity, bias=ip_b[:, 0:1])
                nc.scalar.activation(ZS[:, sl], z_ps[:], Act.Silu,
                                     bias=ip_b[:, 1:2])
            # ---- causal conv1d (4 taps) + silu ----
            # xc_t = sum_k w_k * xm_{t-3+k}; XM0 holds xm at col 4,
            # XM1 at col 3: tap k reads XM0[1+k:] or XM1[k:] — use whichever
            # start offset is even so bf16 ops keep 4B alignment.
            ACC1 = bpool.tile([DI, L], bf16, name="ACC1", tag="ACC1")
            ACC2 = bpool.tile([DI, L], bf16, name="ACC2", tag="ACC2")
            if phases < 2:
                nc.gpsimd.dma_start(out_d[:, 0:CH], SEQ[:, 0:CH])
            if phases >= 2:
                nc.vector.tensor_scalar_mul(ACC1[:], XM0[:, 1:1 + L], cd_w[:, 0:1])
                nc.vector.scalar_tensor_tensor(ACC2[:], XM0[:, 2:2 + L], cd_w[:, 1:2],
                                               ACC1[:], op0=Alu.mult, op1=Alu.add)
                nc.vector.scalar_tensor_tensor(ACC1[:], XM0[:, 3:3 + L], cd_w[:, 2:3],
                                               ACC2[:], op0=Alu.mult, op1=Alu.add)
                nc.vector.scalar_tensor_tensor(ACC2[:], XM0[:, 4:4 + L], cd_w[:, 3:4],
                                               ACC1[:], op0=Alu.mult, op1=Alu.add)
                nc.scalar.activation(XC[:], ACC2[:], Act.Silu, bias=cd_b)

            # ---- x_proj (dt rows) + dt_proj + softplus ----
            for chi in range(NCH if phases >= 2 else 0):
                sl = slice(chi * CH, (chi + 1) * CH)
                dtr_ps = ps.tile([DR, CH], f32, tag="mm")
                nc.tensor.matmul(dtr_ps[:], xpdt_lhsT, XC[:, sl],
                                 start=True, stop=True)
                dtr_sb = spool.tile([DR, CH], bf16, tag="dtrsb")
                nc.scalar.activation(dtr_sb[:], dtr_ps[:], Act.Copy)
                dt_ps = ps.tile([DI, CH], f32, tag="mm")
                nc.tensor.matmul(dt_ps[:], dt_lhsT, dtr_sb[:],
                                 start=True, stop=True)
                esb = spool.tile([DI, CH], f32, tag="esb")
                nc.scalar.activation(esb[:], dt_ps[:], Act.Exp, bias=dt_b)
                nc.scalar.activation(DT[:, sl], esb[:], Act.Ln, bias=1.0)
            if phases >= 2:
                nc.vector.tensor_mul(U[:], DT[:], XC[:])

            # ---- per-state: dA = exp(a_j*dt), dBx = u*B_j, scan ----
            LH = L // 2
            for half in range(2):
                for j in range(NS if phases >= 2.5 else 0):
                    hsl = slice(half * LH, (half + 1) * LH)
                    dA = dapool.tile([DI, LH], f32, tag="dA")
                    nc.scalar.activation(dA[:], DT[:, hsl], Act.Exp,
                                         scale=a_sc[:, j:j + 1])
                    dbx = dbxpool.tile([DI, LH], bf16, tag="dbx")
                    for ci in range(LH // CH):
                        sl = slice(half * LH + ci * CH,
                                   half * LH + (ci + 1) * CH)
                        lsl = slice(ci * CH, (ci + 1) * CH)
                        br = ps.tile([DI, CH], f32, tag="mm")
                        nc.tensor.matmul(br[:], brep_lhsT[:, j * DI:(j + 1) * DI],
                                         XC[:, sl], start=True, stop=True)
                        nc.vector.tensor_tensor(dbx[:, lsl], U[:, sl], br[:],
                                                op=Alu.mult)
                    init = 0.0 if half == 0 else Hs[j][:, LH - 1:LH]
                    nc.vector.tensor_tensor_scan(Hs[j][:, hsl], dA[:], dbx[:],
                                                 init, op0=Alu.mult, op1=Alu.add)

            # ---- y accumulation: y = sum_j H_j * C_j  (PE-accumulated) ----
            y_in_t = dpool.tile([DI, L], bf16, tag="yin")
            y_out_t = dpool.tile([DI, L], bf16, tag="yout")
            for chi in range(NCH if phases >= 3 else 0):
                sl = slice(chi * CH, (chi + 1) * CH)
                yps = psy.tile([DI, CH], f32, tag="yps")
                for j in range(NS):
                    cr = ps.tile([DI, CH], f32, tag="mm")
                    nc.tensor.matmul(cr[:], crep_lhsT[:, j * DI:(j + 1) * DI],
                                     XC[:, sl], start=True, stop=True)
                    tmp = spool.tile([DI, CH], bf16, tag="ymul")
                    nc.vector.tensor_tensor(tmp[:], Hs[j][:, sl], cr[:],
                                            op=Alu.mult)
                    nc.tensor.matmul(yps[:], ident, tmp[:],
                                     start=(j == 0), stop=(j == NS - 1))
                ysb = spool.tile([DI, CH], bf16, tag="ysb")
                nc.scalar.activation(ysb[:], yps[:], Act.Copy)
                nc.sync.dma_start(y_in_t[:, sl], ysb[:])

            # ---- AllReduce partial y within batch pair (2 halves) ----
            if sim or phases < 3:
                nc.sync.dma_start(y_out_t[:], y_in_t[:])
            else:
                nc.gpsimd.collective_compute(
                    "AllReduce", Alu.add, replica_groups=groups,
                    ins=[y_in_t.opt()], outs=[y_out_t.opt()])
            nc.sync.dma_start(YSUM[:], y_out_t[:])

            # ---- post: ys = (y + xc*Dp) * silu(z); out = op(ys) + seq ----
            XCD = bpool.tile([DI, L], bf16, name="XCD", tag="DT")
            YS = bpool.tile([DI, L], bf16, tag="YS")
            for hf in range(2):
                hsl2 = slice(hf * (L // 2), (hf + 1) * (L // 2))
                nc.vector.tensor_scalar_mul(XCD[:, hsl2], XC[:, hsl2], Dp)
                nc.vector.tensor_add(XCD[:, hsl2], YSUM[:, hsl2], XCD[:, hsl2])
                nc.vector.tensor_mul(YS[:, hsl2], XCD[:, hsl2], ZS[:, hsl2])
            OUT = bpool.tile([C, L], f32, name="OUT", tag="XM0")
            for chi in range(NCH):
                sl = slice(chi * CH, (chi + 1) * CH)
                op_ps = ps.tile([C, CH], f32, tag="mm")
                nc.tensor.matmul(op_ps[:], op_lhsT, YS[:, sl],
                                 start=True, stop=True)
                nc.vector.tensor_tensor(OUT[:, sl], op_ps[:], SEQ[:, sl],
                                        op=Alu.add)
                nc.sync.dma_start(out_d[:, sl], OUT[:, sl])

    nc.compile()
    return nc


def _host_precompute(inp):
    import ml_dtypes
    f = lambda k: np.asarray(inp[k], np.float32)
    bf = lambda a: np.ascontiguousarray(a.astype(ml_dtypes.bfloat16))
    w1 = f("conv_w")[:, :, 0, 0]
    wh = f("dwh_w")[:, 0, :, 0]
    ww = f("dww_w")[:, 0, 0, :]
    taps = [
        w1 * (1.0 + wh[:, 1] + ww[:, 1])[None, :],   # center
        w1 * wh[:, 0][None, :],                       # up
        w1 * wh[:, 2][None, :],                       # down
        w1 * ww[:, 0][None, :],                       # left
        w1 * ww[:, 2][None, :],                       # right
    ]
    cw = np.concatenate([t.T for t in taps], axis=1)  # [cin=64, 5*64]
    btot = f("conv_b") + w1 @ (f("dwh_b") + f("dww_b"))
    s_bn = f("bn_g") / np.sqrt(f("bn_v") + EPS)
    bn_bias = s_bn * (btot - f("bn_m")) + f("bn_b")
    ipw = f("in_proj_w")
    ip_lhsT = (ipw * f("ln_g")[None, :]).T            # [64, 256]
    ip_bias = ipw @ f("ln_b")                          # [256]
    xpw = f("x_proj_w")                                # [36, 128]
    a_full = -np.exp(np.asarray(inp["A_log"], np.float32))  # [DI, DS]

    per_sigma = []
    for sg in range(2):
        s_lo = sg * NS
        cf32 = np.zeros((128, 19), np.float32)
        cf32[:C, 0] = s_bn
        cf32[:C, 1] = bn_bias
        cf32[:, 2] = ip_bias[:DI]
        cf32[:, 3] = ip_bias[DI:]
        cf32[:, 4:8] = f("convd_w")[:, 0, :]
        cf32[:, 8] = f("convd_b")
        cf32[:, 9] = f("dt_proj_b")
        for j in range(NS):
            cf32[:, 10 + j] = a_full[:, s_lo + j]
        cf32[:, 18] = f("Dp")

        cbf = np.zeros((128, 2948), np.float32)
        cbf[:, 0:128] = np.eye(128, dtype=np.float32)
        cbf[:C, 128:448] = cw
        cbf[:C, 448:704] = ip_lhsT
        cbf[:, 704:708] = xpw[:DR].T
        cbf[:DR, 708:836] = f("dt_proj_w").T
        for j in range(NS):
            s = s_lo + j
            cbf[:, 836 + j * DI:836 + (j + 1) * DI] = xpw[DR + s][:, None]
            cbf[:, 1860 + j * DI:1860 + (j + 1) * DI] = xpw[DR + DS + s][:, None]
        cbf[:, 2884:2948] = f("out_proj_w").T
        per_sigma.append(dict(cf32=cf32, cbf=bf(cbf)))
    return {}, per_sigma


def _shift_images(xb):
    # 5 pre-shifted copies: ctr, up(reads h-1), dn(h+1), lf(w-1), rt(w+1)
    import ml_dtypes
    out = np.zeros((C, 5, H, W), np.float32)
    out[:, 0] = xb
    out[:, 1, 1:, :] = xb[:, :-1, :]
    out[:, 2, :-1, :] = xb[:, 1:, :]
    out[:, 3, :, 1:] = xb[:, :, :-1]
    out[:, 4, :, :-1] = xb[:, :, 1:]
    return np.ascontiguousarray(
        out.transpose(1, 0, 2, 3).reshape(5, C, L).transpose(1, 0, 2)
        .reshape(C, 5 * L).astype(ml_dtypes.bfloat16))


TRACE = False
LAST_EXEC_NS = None
LAST_TRACE_DIR = None


def kernel(**inputs):
    global LAST_EXEC_NS, LAST_TRACE_DIR
    from concourse.bass_utils import run_bass_kernel_spmd

    if "nc" not in _cached:
        _cached["nc"] = _build_program()
    nc = _cached["nc"]

    common, per_sigma = _host_precompute(inputs)
    x = np.asarray(inputs["x"], np.float32)
    in_maps = []
    for c in range(NCORES):
        b, sg = c // 2, c % 2
        m = dict(common)
        m.update(per_sigma[sg])
        m["ximgs"] = _shift_images(x[b])
        in_maps.append(m)

    kw = {}
    if TRACE:
        import tempfile
        LAST_TRACE_DIR = tempfile.mkdtemp(prefix="bass_trace_")
        kw = dict(trace=True, tmpdir=LAST_TRACE_DIR)
    r = run_bass_kernel_spmd(nc, in_maps, list(range(NCORES)), **kw)
    if r.exec_time_ns is not None:
        LAST_EXEC_NS = r.exec_time_ns
    res = r.results
    out = np.empty((B, C, H, W), np.float32)
    for b in range(B):
        out[b] = np.asarray(res[2 * b]["out_f"], np.float32).reshape(C, H, W)
    return out



# revision 8
# speedup vs baseline: 1.4616x; 1.4616x over previous
"""Trainium2 Bass kernel v2 for nn_DecoderBlock_Mamba.

Sharding: 8 cores = (batch b in 0..3) x (state-half sigma in {0,1}).
Scan uses a tiled partition layout: partition p = j*16 + i holds state
(s_lo + j) and channel-group offset i; 8 channel-groups g cover d = 16g + i.
This makes the B/C broadcasts group-invariant (built once) and both dbx / y
multiplies all-SBUF-bf16 (2x DVE fast path). U/DT are staged to DRAM and
replicated into the tiled layout by 8 DMAs per group (DMA engines are idle).

Self-contained: hardcodes all shapes; no sibling imports.
"""
import numpy as np

C = 64
DI = 128
DS = 16
DR = 4
B = 4
H = 64
W = 64
L = H * W
NS = 8            # states per core
NG = 8            # channel groups (of 16) per core
NCORES = 8
NCH = 8           # L chunks of 512
CH = 512
EPS = 1e-5

_cached = {}


def _build_program(sim=False, phases=3):
    import concourse.bass as bass
    import concourse.bacc as bacc
    import concourse.mybir as mybir
    import concourse.tile as tile

    dt = mybir.dt
    f32 = dt.float32
    bf16 = dt.bfloat16
    Act = mybir.ActivationFunctionType
    Alu = mybir.AluOpType
    Axis = mybir.AxisListType

    nc = bacc.Bacc(None, target_bir_lowering=False)

    def din(name, shape, dtype=f32):
        return nc.dram_tensor(name, shape, dtype, kind="ExternalInput")

    ximgs_d = din("ximgs", [C, 5 * L], bf16)
    cf32_d = din("cf32", [128, 32])
    cbf_d = din("cbf", [128, 2688], bf16)

    out_d = nc.dram_tensor("out_f", [C, L], f32, kind="ExternalOutput")

    groups = [[0, 1], [2, 3], [4, 5], [6, 7]]

    with tile.TileContext(nc) as tc:
        with (
            tc.tile_pool(name="dram", bufs=1, space="DRAM") as dpool,
            tc.tile_pool(name="const", bufs=1) as cpool,
            tc.tile_pool(name="big", bufs=1) as bpool,
            tc.tile_pool(name="sm", bufs=2) as spool,
            tc.tile_pool(name="ud", bufs=2) as udpool,
            tc.tile_pool(name="da", bufs=2) as dapool,
            tc.tile_pool(name="ps", bufs=4, space="PSUM") as ps,
            tc.tile_pool(name="psy", bufs=4, space="PSUM") as psy,
        ):
            # ---- constants (packed: 2 DMAs) ----
            cf = cpool.tile([128, 32], f32)
            cb = cpool.tile([128, 2688], bf16)
            nc.sync.dma_start(cf[:], cf32_d[:])
            nc.sync.dma_start(cb[:], cbf_d[:])
            bn_s = cf[0:C, 0:1]
            bn_b = cf[0:C, 1:2]
            ip_b0 = cf[:, 2:3]
            ip_b1 = cf[:, 3:4]
            cd_b = cf[:, 4:5]
            dt_b = cf[:, 5:6]
            Dp = cf[:, 6:7]
            a_vec = cf[:, 8:16]          # per-group a scale [128, 8]

            ident = cb[:, 0:128]
            cw = cb[0:C, 128:448]
            ip_lhsT = cb[0:C, 448:704]
            M_dt = cb[:, 704:832]
            cdiag = cb[:, 832:1344]      # 4 diag taps [128, 4*128]
            W_B = cb[:, 1344:1472]       # fused B broadcast [128,128]
            W_C = cb[:, 1472:1600]       # fused C broadcast
            Rg = cb[:, 1600:2624]        # 8 x [128,128] reduce mats
            op_lhsT = cb[:, 2624:2688]

            # ---- persistent activations ----
            SEQ = bpool.tile([C, L], bf16)            # BN+ReLU out (residual)
            HN = bpool.tile([C, L], bf16)             # LN-normalized
            XM0 = bpool.tile([DI, L + 4], bf16, name="XM0", tag="YP5")       # conv1d in, data @ col 4
            ZS = bpool.tile([DI, L], bf16)            # silu(z)
            XC = bpool.tile([DI, L], bf16)
            ESB = bpool.tile([DI, L], bf16, tag="ESB")
            DT = bpool.tile([DI, L], bf16, name="DT", tag="YSUM")
            U = bpool.tile([DI, L], bf16, name="U", tag="ESB")
            BT = bpool.tile([DI, L], bf16, name="BT")      # B_tile (j slow)
            CT = bpool.tile([DI, L], bf16, name="CT")      # C_tile
            YPs = [bpool.tile([DI, L], bf16, name=f"YP{g}", tag=f"YP{g}")
                   for g in range(NG)]
            YSUM = bpool.tile([DI, L], bf16, name="YSUM", tag="YSUM")

            # staging DRAM for U/DT tiled reads
            ud_dram = dpool.tile([DI, 2 * L], bf16, tag="uddram")
            y_in_t = dpool.tile([4, DI, L // 4], bf16, tag="yin")
            y_out_t = dpool.tile([4, DI, L // 4], bf16, tag="yout")

            # Prime ACT's vector clock on the const DMAs
            warm = cpool.tile([128, 1], f32, tag="warm")
            nc.scalar.activation(warm[:], cf[:, 0:1], Act.Copy)
            warm2 = cpool.tile([128, 1], bf16, tag="warm2")
            nc.scalar.activation(warm2[:], cb[:, 0:1], Act.Copy)
            nc.vector.tensor_scalar_mul(XM0[:, 0:4], cf[:, 0:4], 0.0)

            IMGS = [bpool.tile([C, L], bf16, name=f"img{t}", tag=f"YP{t}")
                    for t in range(5)]
            for t in range(5):
                nc.sync.dma_start(IMGS[t][:], ximgs_d[:, t * L:(t + 1) * L])

            # ---- front conv: 5 accumulating taps + BN + ReLU ----
            for chi in range(NCH):
                sl = slice(chi * CH, (chi + 1) * CH)
                pc = ps.tile([C, CH], f32, tag="mm")
                for tap in range(5):
                    nc.tensor.matmul(pc[:], cw[:, tap * C:(tap + 1) * C],
                                     IMGS[tap][:, sl],
                                     start=(tap == 0), stop=(tap == 4))
                nc.scalar.activation(SEQ[:, sl], pc[:],
                                     Act.Relu, bias=bn_b, scale=bn_s)

            # ---- LayerNorm over channels, batched 4 blocks per op ----
            HN0 = bpool.tile([128, L // 2], bf16, name="HN0", tag="HN0")
            VARS = spool.tile([128, 32], f32, tag="VARS")
            for g in range(NCH):
                tps4 = ps.tile([128, 4, C], bf16, tag="mm")
                for k in range(4):
                    blk = g * 4 + k
                    nc.tensor.transpose(tps4[:, k, :],
                                        SEQ[:, blk * 128:(blk + 1) * 128],
                                        ident[0:C, 0:C])
                mu4 = spool.tile([128, 4], f32, tag="mu4")
                nc.vector.tensor_reduce(mu4[:], tps4[:], Axis.X, Alu.add)
                mun4 = spool.tile([128, 4], f32, tag="mun4")
                nc.vector.tensor_scalar_mul(mun4[:], mu4[:], 1.0 / C)
                h04 = HN0[:, g * 256:(g + 1) * 256].rearrange(
                    "p (b c) -> p b c", b=4)
                nc.vector.tensor_tensor(h04, tps4[:],
                                        mun4[:].to_broadcast((128, 4, C)),
                                        op=Alu.subtract)
                sq4 = spool.tile([128, 4, C], f32, tag="sq4")
                nc.gpsimd.tensor_tensor(sq4[:], h04, h04, op=Alu.mult)
                ssq4 = spool.tile([128, 4], f32, tag="ssq4")
                nc.vector.tensor_reduce(ssq4[:], sq4[:], Axis.X, Alu.add)
                nc.vector.tensor_scalar(VARS[:, g * 4:(g + 1) * 4], ssq4[:],
                                        1.0 / C, EPS,
                                        op0=Alu.mult, op1=Alu.add)
            SQV = spool.tile([128, 32], f32, tag="SQV")
            RSTD = spool.tile([128, 32], f32, tag="RSTD")
            nc.scalar.activation(SQV[:], VARS[:], Act.Sqrt)
            nc.vector.reciprocal(RSTD[:], SQV[:])
            HNT = bpool.tile([128, L // 2], bf16, name="HNT", tag="HNT")
            for g in range(NCH):
                hnT4 = HNT[:, g * 256:(g + 1) * 256].rearrange(
                    "p (b c) -> p b c", b=4)
                nc.gpsimd.tensor_tensor(
                    hnT4, HN0[:, g * 256:(g + 1) * 256].rearrange(
                        "p (b c) -> p b c", b=4),
                    RSTD[:, g * 4:(g + 1) * 4].to_broadcast((128, 4, C)),
                    op=Alu.mult)
                tb4 = ps.tile([C, 4, 128], bf16, tag="mm")
                for k in range(4):
                    blk = g * 4 + k
                    nc.tensor.transpose(tb4[:, k, :],
                                        HNT[:, blk * C:(blk + 1) * C],
                                        ident)
                nc.vector.tensor_scalar(
                    HN[:, g * CH:(g + 1) * CH],
                    tb4[:].rearrange("p a b -> p (a b)"), 0.0,
                    None, op0=Alu.add)

            # ---- in_proj: xm (DVE bias-add) + z (ACT silu) ----
            for chi in range(NCH):
                sl = slice(chi * CH, (chi + 1) * CH)
                xm_ps = ps.tile([DI, CH], f32, tag="mm")
                z_ps = ps.tile([DI, CH], f32, tag="mm")
                nc.tensor.matmul(xm_ps[:], ip_lhsT[0:C, 0:DI], HN[:, sl],
                                 start=True, stop=True)
                nc.tensor.matmul(z_ps[:], ip_lhsT[0:C, DI:2 * DI], HN[:, sl],
                                 start=True, stop=True)
                nc.vector.tensor_scalar(XM0[:, 4 + chi * CH:4 + (chi + 1) * CH],
                                        xm_ps[:], ip_b0, None, op0=Alu.add)
                nc.scalar.activation(ZS[:, sl], z_ps[:], Act.Silu, bias=ip_b1)

            # ---- causal conv1d on PE (4 diag taps) + silu ----
            for chi in range(NCH):
                sl = slice(chi * CH, (chi + 1) * CH)
                cc = ps.tile([DI, CH], f32, tag="mm")
                for tap in range(4):
                    nc.tensor.matmul(cc[:], cdiag[:, tap * 128:(tap + 1) * 128],
                                     XM0[:, 1 + tap + chi * CH:
                                         1 + tap + chi * CH + CH],
                                     start=(tap == 0), stop=(tap == 3))
                nc.scalar.activation(XC[:, sl], cc[:], Act.Silu, bias=cd_b)

            # ---- x_proj: fused dt matmul; B/C rows; esb exp ----
            for chi in range(NCH):
                sl = slice(chi * CH, (chi + 1) * CH)
                dt_ps = ps.tile([DI, CH], f32, tag="mm")
                nc.tensor.matmul(dt_ps[:], M_dt, XC[:, sl],
                                 start=True, stop=True)
                nc.scalar.activation(ESB[:, sl], dt_ps[:], Act.Exp, bias=dt_b)


            # ---- DT = ln(1+esb) in halves (costs ~2 extra act-table loads
            # but lets U/staging start at the front's midpoint);
            # U = DT*XC; stage U/DT to DRAM for tiled replication ----
            LHf = L // 2
            for hf in range(2):
                hsl = slice(hf * LHf, (hf + 1) * LHf)
                nc.scalar.activation(DT[:, hsl], ESB[:, hsl], Act.Ln, bias=1.0)
                nc.vector.tensor_mul(U[:, hsl], DT[:, hsl], XC[:, hsl])
                nc.sync.dma_start(ud_dram[:, hf * LHf:(hf + 1) * LHf],
                                  U[:, hsl])
                nc.sync.dma_start(ud_dram[:, L + hf * LHf:L + (hf + 1) * LHf],
                                  DT[:, hsl])

            # ---- B_tile / C_tile (group-invariant): sel matmul + copy ----
            for chi in range(NCH):
                sl = slice(chi * CH, (chi + 1) * CH)
                bt_ps = ps.tile([DI, CH], f32, tag="mm")
                nc.tensor.matmul(bt_ps[:], W_B, XC[:, sl],
                                 start=True, stop=True)
                nc.scalar.activation(BT[:, sl], bt_ps[:], Act.Copy)
                ct_ps = ps.tile([DI, CH], f32, tag="mm")
                nc.tensor.matmul(ct_ps[:], W_C, XC[:, sl],
                                 start=True, stop=True)
                nc.vector.tensor_scalar(CT[:, sl], ct_ps[:], 0.0, None,
                                        op0=Alu.add)

            # ---- XCD = XC*Dp on Pool (runs during scan phase) ----
            XCD = bpool.tile([DI, L], bf16, name="XCD", tag="ESB")
            for hf in range(2):
                hsl = slice(hf * (L // 2), (hf + 1) * (L // 2))
                nc.gpsimd.tensor_tensor(XCD[:, hsl], XC[:, hsl],
                                        Dp.to_broadcast((DI, L // 2)),
                                        op=Alu.mult)

            # ---- scan phase: per channel-group g ----
            LH0 = L // 2
            ypsA = [psy.tile([DI, CH], f32, name=f"ypsA{ci}", tag="yps")
                    for ci in range(4)]
            for g in range(NG):
                udt = udpool.tile([DI, 2 * L], bf16, tag="udt")
                for j in range(NS):
                    nc.sync.dma_start(
                        udt[j * 16:(j + 1) * 16, :],
                        ud_dram[g * 16:(g + 1) * 16, :])
                for hf in range(2):
                    hsl = slice(hf * LH0, (hf + 1) * LH0)
                    dA = dapool.tile([DI, LH0], f32, tag="dA")
                    nc.scalar.activation(dA[:],
                                         udt[:, L + hf * LH0:L + (hf + 1) * LH0],
                                         Act.Exp, scale=a_vec[:, g:g + 1])
                    # dbx in-place into the U-half of udt (dead after this)
                    nc.vector.tensor_tensor(udt[:, hsl], udt[:, hsl],
                                            BT[:, hsl], op=Alu.mult)
                    init = 0.0 if hf == 0 else YPs[g][:, LH0 - 1:LH0]
                    nc.vector.tensor_tensor_scan(YPs[g][:, hsl], dA[:],
                                                 udt[:, hsl],
                                                 init, op0=Alu.mult, op1=Alu.add)
                # y partial: YP = H * C_tile (even groups on Pool)
                if g % 2 == 0:
                    nc.gpsimd.tensor_tensor(YPs[g][:], YPs[g][:], CT[:],
                                            op=Alu.mult)
                else:
                    nc.vector.tensor_tensor(YPs[g][:], YPs[g][:], CT[:],
                                            op=Alu.mult)
                # incremental y-reduce for chunks 0-3 (PSUM live across phase)
                for ci in range(4):
                    slc = slice(ci * CH, (ci + 1) * CH)
                    nc.tensor.matmul(ypsA[ci][:], Rg[:, g * 128:(g + 1) * 128],
                                     YPs[g][:, slc],
                                     start=(g == 0), stop=(g == NG - 1))

            # ---- y reduce: chunks 0-3 done incrementally; drain + chunks 4-7
            for chi in range(4):
                sl = slice(chi * CH, (chi + 1) * CH)
                ysb = spool.tile([DI, CH], bf16, tag="ysb")
                nc.scalar.activation(ysb[:], ypsA[chi][:], Act.Copy)
                nc.sync.dma_start(
                    y_in_t[chi // 2, :, (chi % 2) * CH:(chi % 2 + 1) * CH],
                    ysb[:])
            for chi in range(4, NCH):
                sl = slice(chi * CH, (chi + 1) * CH)
                yps = psy.tile([DI, CH], f32, tag="yps")
                for g in range(NG):
                    nc.tensor.matmul(yps[:], Rg[:, g * 128:(g + 1) * 128],
                                     YPs[g][:, sl],
                                     start=(g == 0), stop=(g == NG - 1))
                ysb = spool.tile([DI, CH], bf16, tag="ysb")
                if chi % 2 == 0:
                    nc.scalar.activation(ysb[:], yps[:], Act.Copy)
                else:
                    nc.vector.tensor_scalar(ysb[:], yps[:], 0.0, None,
                                            op0=Alu.add)
                nc.sync.dma_start(
                    y_in_t[chi // 2, :, (chi % 2) * CH:(chi % 2 + 1) * CH],
                    ysb[:])

            # ---- AllReduce partial y in quarters, pipelined with post/out ----
            # XCD precomputed on Pool (overlaps scan phase)
            YS = bpool.tile([DI, L], bf16, name="YS", tag="HN")
            LQ = L // 4
            for q in range(4):
                qsl = slice(q * LQ, (q + 1) * LQ)
                if sim:
                    nc.sync.dma_start(y_out_t[q], y_in_t[q])
                else:
                    nc.gpsimd.collective_compute(
                        "AllReduce", Alu.add, replica_groups=groups,
                        ins=[y_in_t[q].opt()], outs=[y_out_t[q].opt()])
                nc.sync.dma_start(YSUM[:, qsl], y_out_t[q])
                nc.vector.tensor_add(XCD[:, qsl], YSUM[:, qsl], XCD[:, qsl])
                nc.vector.tensor_mul(YS[:, qsl], XCD[:, qsl], ZS[:, qsl])
                for ci in range(2):
                    chi = q * 2 + ci
                    sl = slice(chi * CH, (chi + 1) * CH)
                    op_ps = psy.tile([C, CH], f32, tag="yps")
                    nc.tensor.matmul(op_ps[:], op_lhsT, YS[:, sl],
                                     start=True, stop=False)
                    nc.tensor.matmul(op_ps[:], ident[0:C, 0:C], SEQ[:, sl],
                                     start=False, stop=True)
                    outc = spool.tile([C, CH], f32, tag="outc")
                    if ci == 0:
                        nc.vector.tensor_scalar(outc[:], op_ps[:], 0.0, None,
                                                op0=Alu.add)
                    else:
                        nc.scalar.activation(outc[:], op_ps[:], Act.Copy)
                    nc.sync.dma_start(out_d[:, sl], outc[:])

    nc.compile()
    return nc


def _host_precompute(inp):
    import ml_dtypes
    f = lambda k: np.asarray(inp[k], np.float32)
    bf = lambda a: np.ascontiguousarray(a.astype(ml_dtypes.bfloat16))
    w1 = f("conv_w")[:, :, 0, 0]
    wh = f("dwh_w")[:, 0, :, 0]
    ww = f("dww_w")[:, 0, 0, :]
    taps = [
        w1 * (1.0 + wh[:, 1] + ww[:, 1])[None, :],   # center
        w1 * wh[:, 0][None, :],                       # up
        w1 * wh[:, 2][None, :],                       # down
        w1 * ww[:, 0][None, :],                       # left
        w1 * ww[:, 2][None, :],                       # right
    ]
    cw = np.concatenate([t.T for t in taps], axis=1)  # [cin=64, 5*64]
    btot = f("conv_b") + w1 @ (f("dwh_b") + f("dww_b"))
    s_bn = f("bn_g") / np.sqrt(f("bn_v") + EPS)
    bn_bias = s_bn * (btot - f("bn_m")) + f("bn_b")
    ipw = f("in_proj_w")
    ip_lhsT = (ipw * f("ln_g")[None, :]).T            # [64, 256]
    ip_bias = ipw @ f("ln_b")                          # [256]
    xpw = f("x_proj_w")                                # [36, 128]
    M_dt = f("dt_proj_w") @ xpw[:DR]                   # [128, 128]
    a_full = -np.exp(np.asarray(inp["A_log"], np.float32))  # [DI, DS]
    cdw = f("convd_w")[:, 0, :]                        # [128, 4]

    per_sigma = []
    for sg in range(2):
        s_lo = sg * NS
        cf32 = np.zeros((128, 32), np.float32)
        cf32[:C, 0] = s_bn
        cf32[:C, 1] = bn_bias
        cf32[:, 2] = ip_bias[:DI]
        cf32[:, 3] = ip_bias[DI:]
        cf32[:, 4] = f("convd_b")
        cf32[:, 5] = f("dt_proj_b")
        cf32[:, 6] = f("Dp")
        # a_vec per group g: a[p] = a_full[16g + p%16, s_lo + p//16]
        p = np.arange(128)
        for g in range(NG):
            cf32[:, 8 + g] = a_full[16 * g + p % 16, s_lo + p // 16]

        cbf = np.zeros((128, 2688), np.float32)
        cbf[:, 0:128] = np.eye(128, dtype=np.float32)
        cbf[:C, 128:448] = cw
        cbf[:C, 448:704] = ip_lhsT
        cbf[:, 704:832] = M_dt.T
        for tap in range(4):
            cbf[:, 832 + tap * 128:832 + (tap + 1) * 128] = np.diag(cdw[:, tap])
        # fused B/C broadcast: W_B[p, :] = xpw_B[s_lo + p//16, :] (stored T)
        for pp in range(128):
            cbf[:, 1344 + pp] = xpw[DR + s_lo + pp // 16]
            cbf[:, 1472 + pp] = xpw[DR + DS + s_lo + pp // 16]
        # Rg: R_g[p, d] = 1 iff d == 16g + p%16
        for g in range(NG):
            for pp in range(128):
                cbf[pp, 1600 + g * 128 + 16 * g + pp % 16] = 1.0
        cbf[:, 2624:2688] = f("out_proj_w").T
        per_sigma.append(dict(cf32=cf32, cbf=bf(cbf)))
    return {}, per_sigma


def _shift_images(xb):
    # 5 pre-shifted copies: ctr, up(reads h-1), dn(h+1), lf(w-1), rt(w+1)
    import ml_dtypes
    out = np.zeros((C, 5, H, W), np.float32)
    out[:, 0] = xb
    out[:, 1, 1:, :] = xb[:, :-1, :]
    out[:, 2, :-1, :] = xb[:, 1:, :]
    out[:, 3, :, 1:] = xb[:, :, :-1]
    out[:, 4, :, :-1] = xb[:, :, 1:]
    return np.ascontiguousarray(
        out.transpose(1, 0, 2, 3).reshape(5, C, L).transpose(1, 0, 2)
        .reshape(C, 5 * L).astype(ml_dtypes.bfloat16))


TRACE = False
LAST_EXEC_NS = None
LAST_TRACE_DIR = None


def kernel(**inputs):
    global LAST_EXEC_NS, LAST_TRACE_DIR
    from concourse.bass_utils import run_bass_kernel_spmd

    if "nc" not in _cached:
        _cached["nc"] = _build_program()
    nc = _cached["nc"]

    common, per_sigma = _host_precompute(inputs)
    x = np.asarray(inputs["x"], np.float32)
    in_maps = []
    for c in range(NCORES):
        b, sg = c // 2, c % 2
        m = dict(common)
        m.update(per_sigma[sg])
        m["ximgs"] = _shift_images(x[b])
        in_maps.append(m)

    kw = {}
    if TRACE:
        import tempfile
        LAST_TRACE_DIR = tempfile.mkdtemp(prefix="bass_trace_")
        kw = dict(trace=True, tmpdir=LAST_TRACE_DIR)
    r = run_bass_kernel_spmd(nc, in_maps, list(range(NCORES)), **kw)
    if r.exec_time_ns is not None:
        LAST_EXEC_NS = r.exec_time_ns
    res = r.results
    out = np.empty((B, C, H, W), np.float32)
    for b in range(B):
        out[b] = np.asarray(res[2 * b]["out_f"], np.float32).reshape(C, H, W)
    return out


# revision 12
# speedup vs baseline: 1.4803x; 1.0129x over previous
"""Trainium2 Bass kernel v2 for nn_DecoderBlock_Mamba.

Sharding: 8 cores = (batch b in 0..3) x (state-half sigma in {0,1}).
Scan uses a tiled partition layout: partition p = j*16 + i holds state
(s_lo + j) and channel-group offset i; 8 channel-groups g cover d = 16g + i.
This makes the B/C broadcasts group-invariant (built once) and both dbx / y
multiplies all-SBUF-bf16 (2x DVE fast path). U/DT are staged to DRAM and
replicated into the tiled layout by 8 DMAs per group (DMA engines are idle).

Self-contained: hardcodes all shapes; no sibling imports.
"""
import numpy as np

C = 64
DI = 128
DS = 16
DR = 4
B = 4
H = 64
W = 64
L = H * W
NS = 8            # states per core
NG = 8            # channel groups (of 16) per core
NCORES = 8
NCH = 8           # L chunks of 512
CH = 512
EPS = 1e-5

_cached = {}


def _build_program(sim=False, phases=3):
    import concourse.bass as bass
    import concourse.bacc as bacc
    import concourse.mybir as mybir
    import concourse.tile as tile

    dt = mybir.dt
    f32 = dt.float32
    bf16 = dt.bfloat16
    Act = mybir.ActivationFunctionType
    Alu = mybir.AluOpType
    Axis = mybir.AxisListType

    nc = bacc.Bacc(None, target_bir_lowering=False)

    def din(name, shape, dtype=f32):
        return nc.dram_tensor(name, shape, dtype, kind="ExternalInput")

    ximgs_d = din("ximgs", [C, 5 * L], bf16)
    cf32_d = din("cf32", [128, 32])
    cbf_d = din("cbf", [128, 2688], bf16)

    out_d = nc.dram_tensor("out_f", [C, L], f32, kind="ExternalOutput")

    groups = [[0, 1], [2, 3], [4, 5], [6, 7]]

    with tile.TileContext(nc) as tc:
        with (
            tc.tile_pool(name="dram", bufs=1, space="DRAM") as dpool,
            tc.tile_pool(name="const", bufs=1) as cpool,
            tc.tile_pool(name="big", bufs=1) as bpool,
            tc.tile_pool(name="sm", bufs=2) as spool,
            tc.tile_pool(name="ud", bufs=2) as udpool,
            tc.tile_pool(name="da", bufs=2) as dapool,
            tc.tile_pool(name="ps", bufs=4, space="PSUM") as ps,
            tc.tile_pool(name="psy", bufs=4, space="PSUM") as psy,
        ):
            # ---- constants (packed: 2 DMAs) ----
            cf = cpool.tile([128, 32], f32)
            cb = cpool.tile([128, 2688], bf16)
            nc.sync.dma_start(cf[:], cf32_d[:])
            nc.sync.dma_start(cb[:], cbf_d[:])
            bn_s = cf[0:C, 0:1]
            bn_b = cf[0:C, 1:2]
            ip_b0 = cf[:, 2:3]
            ip_b1 = cf[:, 3:4]
            cd_b = cf[:, 4:5]
            dt_b = cf[:, 5:6]
            Dp = cf[:, 6:7]
            a_vec = cf[:, 8:16]          # per-group a scale [128, 8]

            ident = cb[:, 0:128]
            cw = cb[0:C, 128:448]
            ip_lhsT = cb[0:C, 448:704]
            M_dt = cb[:, 704:832]
            cdiag = cb[:, 832:1344]      # 4 diag taps [128, 4*128]
            W_B = cb[:, 1344:1472]       # fused B broadcast [128,128]
            W_C = cb[:, 1472:1600]       # fused C broadcast
            Rg = cb[:, 1600:2624]        # 8 x [128,128] reduce mats
            op_lhsT = cb[:, 2624:2688]

            # ---- persistent activations ----
            SEQ = bpool.tile([C, L], bf16)            # BN+ReLU out (residual)
            HN = bpool.tile([C, L], bf16)             # LN-normalized
            XM0 = bpool.tile([DI, L + 4], bf16, name="XM0", tag="YP5")       # conv1d in, data @ col 4
            ZS = bpool.tile([DI, L], bf16)            # silu(z)
            XC = bpool.tile([DI, L], bf16)
            ESB = bpool.tile([DI, L], bf16, tag="ESB")
            DT = bpool.tile([DI, L], bf16, name="DT", tag="YSUM")
            U = bpool.tile([DI, L], bf16, name="U", tag="ESB")
            BT = bpool.tile([DI, L], bf16, name="BT")      # B_tile (j slow)
            CT = bpool.tile([DI, L], bf16, name="CT")      # C_tile
            YPs = [bpool.tile([DI, L], bf16, name=f"YP{g}", tag=f"YP{g}")
                   for g in range(NG)]
            YSUM = bpool.tile([DI, L], bf16, name="YSUM", tag="YSUM")

            # staging DRAM for U/DT tiled reads
            ud_dram = dpool.tile([DI, 2 * L], bf16, tag="uddram")
            y_in_t = dpool.tile([4, DI, L // 4], bf16, tag="yin")
            y_out_t = dpool.tile([4, DI, L // 4], bf16, tag="yout")

            # Prime ACT's vector clock on the const DMAs
            warm = cpool.tile([128, 1], f32, tag="warm")
            nc.scalar.activation(warm[:], cf[:, 0:1], Act.Copy)
            warm2 = cpool.tile([128, 1], bf16, tag="warm2")
            nc.scalar.activation(warm2[:], cb[:, 0:1], Act.Copy)
            nc.vector.tensor_scalar_mul(XM0[:, 0:4], cf[:, 0:4], 0.0)

            IMGS = [bpool.tile([C, L], bf16, name=f"img{t}", tag=f"YP{t}")
                    for t in range(5)]
            for t in range(5):
                nc.sync.dma_start(IMGS[t][:], ximgs_d[:, t * L:(t + 1) * L])

            # ---- front conv: 5 accumulating taps + BN + ReLU ----
            for chi in range(NCH):
                sl = slice(chi * CH, (chi + 1) * CH)
                pc = ps.tile([C, CH], f32, tag="mm")
                for tap in range(5):
                    nc.tensor.matmul(pc[:], cw[:, tap * C:(tap + 1) * C],
                                     IMGS[tap][:, sl],
                                     start=(tap == 0), stop=(tap == 4))
                nc.scalar.activation(SEQ[:, sl], pc[:],
                                     Act.Relu, bias=bn_b, scale=bn_s)

            # ---- LayerNorm over channels, batched 4 blocks per op ----
            HN0 = bpool.tile([128, L // 2], bf16, name="HN0", tag="HN0")
            VARS = spool.tile([128, 32], f32, tag="VARS")
            SQV = spool.tile([128, 32], f32, tag="SQV")
            RSTD = spool.tile([128, 32], f32, tag="RSTD")
            for g in range(NCH):
                tps4 = ps.tile([128, 4, C], bf16, tag="mm")
                for k in range(4):
                    blk = g * 4 + k
                    nc.tensor.transpose(tps4[:, k, :],
                                        SEQ[:, blk * 128:(blk + 1) * 128],
                                        ident[0:C, 0:C])
                mu4 = spool.tile([128, 4], f32, tag="mu4")
                nc.vector.tensor_reduce(mu4[:], tps4[:], Axis.X, Alu.add)
                mun4 = spool.tile([128, 4], f32, tag="mun4")
                nc.vector.tensor_scalar_mul(mun4[:], mu4[:], 1.0 / C)
                h04 = HN0[:, g * 256:(g + 1) * 256].rearrange(
                    "p (b c) -> p b c", b=4)
                nc.vector.tensor_tensor(h04, tps4[:],
                                        mun4[:].to_broadcast((128, 4, C)),
                                        op=Alu.subtract)
                sq4 = spool.tile([128, 4, C], f32, tag="sq4")
                nc.gpsimd.tensor_tensor(sq4[:], h04, h04, op=Alu.mult)
                ssq4 = spool.tile([128, 4], f32, tag="ssq4")
                nc.vector.tensor_reduce(ssq4[:], sq4[:], Axis.X, Alu.add)
                nc.vector.tensor_scalar(VARS[:, g * 4:(g + 1) * 4], ssq4[:],
                                        1.0 / C, EPS,
                                        op0=Alu.mult, op1=Alu.add)
                nc.scalar.activation(SQV[:, g * 4:(g + 1) * 4],
                                     VARS[:, g * 4:(g + 1) * 4], Act.Sqrt)
                nc.vector.reciprocal(RSTD[:, g * 4:(g + 1) * 4],
                                     SQV[:, g * 4:(g + 1) * 4])
            HNT = bpool.tile([128, L // 2], bf16, name="HNT", tag="HNT")
            for g in range(NCH):
                hnT4 = HNT[:, g * 256:(g + 1) * 256].rearrange(
                    "p (b c) -> p b c", b=4)
                nc.gpsimd.tensor_tensor(
                    hnT4, HN0[:, g * 256:(g + 1) * 256].rearrange(
                        "p (b c) -> p b c", b=4),
                    RSTD[:, g * 4:(g + 1) * 4].to_broadcast((128, 4, C)),
                    op=Alu.mult)
                tb4 = ps.tile([C, 4, 128], bf16, tag="mm")
                for k in range(4):
                    blk = g * 4 + k
                    nc.tensor.transpose(tb4[:, k, :],
                                        HNT[:, blk * C:(blk + 1) * C],
                                        ident)
                nc.vector.tensor_scalar(
                    HN[:, g * CH:(g + 1) * CH],
                    tb4[:].rearrange("p a b -> p (a b)"), 0.0,
                    None, op0=Alu.add)

            # ---- in_proj: xm (DVE bias-add) + z (ACT silu) ----
            for chi in range(NCH):
                sl = slice(chi * CH, (chi + 1) * CH)
                xm_ps = ps.tile([DI, CH], f32, tag="mm")
                z_ps = ps.tile([DI, CH], f32, tag="mm")
                nc.tensor.matmul(xm_ps[:], ip_lhsT[0:C, 0:DI], HN[:, sl],
                                 start=True, stop=True)
                nc.tensor.matmul(z_ps[:], ip_lhsT[0:C, DI:2 * DI], HN[:, sl],
                                 start=True, stop=True)
                nc.vector.tensor_scalar(XM0[:, 4 + chi * CH:4 + (chi + 1) * CH],
                                        xm_ps[:], ip_b0, None, op0=Alu.add)
                nc.scalar.activation(ZS[:, sl], z_ps[:], Act.Silu, bias=ip_b1)

            # ---- causal conv1d on PE (4 diag taps) + silu ----
            for chi in range(NCH):
                sl = slice(chi * CH, (chi + 1) * CH)
                cc = ps.tile([DI, CH], f32, tag="mm")
                for tap in range(4):
                    nc.tensor.matmul(cc[:], cdiag[:, tap * 128:(tap + 1) * 128],
                                     XM0[:, 1 + tap + chi * CH:
                                         1 + tap + chi * CH + CH],
                                     start=(tap == 0), stop=(tap == 3))
                nc.scalar.activation(XC[:, sl], cc[:], Act.Silu, bias=cd_b)

            # ---- x_proj: fused dt matmul; B/C rows; esb exp ----
            for chi in range(NCH):
                sl = slice(chi * CH, (chi + 1) * CH)
                dt_ps = ps.tile([DI, CH], f32, tag="mm")
                nc.tensor.matmul(dt_ps[:], M_dt, XC[:, sl],
                                 start=True, stop=True)
                nc.scalar.activation(ESB[:, sl], dt_ps[:], Act.Exp, bias=dt_b)


            # ---- DT = ln(1+esb) in halves (costs ~2 extra act-table loads
            # but lets U/staging start at the front's midpoint);
            # U = DT*XC; stage U/DT to DRAM for tiled replication ----
            LHf = L // 2
            for hf in range(2):
                hsl = slice(hf * LHf, (hf + 1) * LHf)
                nc.scalar.activation(DT[:, hsl], ESB[:, hsl], Act.Ln, bias=1.0)
                nc.vector.tensor_mul(U[:, hsl], DT[:, hsl], XC[:, hsl])
                nc.sync.dma_start(ud_dram[:, hf * LHf:(hf + 1) * LHf],
                                  U[:, hsl])
                nc.sync.dma_start(ud_dram[:, L + hf * LHf:L + (hf + 1) * LHf],
                                  DT[:, hsl])

            # ---- B_tile / C_tile (group-invariant): sel matmul + copy ----
            for chi in range(NCH):
                sl = slice(chi * CH, (chi + 1) * CH)
                bt_ps = ps.tile([DI, CH], f32, tag="mm")
                nc.tensor.matmul(bt_ps[:], W_B, XC[:, sl],
                                 start=True, stop=True)
                nc.scalar.activation(BT[:, sl], bt_ps[:], Act.Copy)
                ct_ps = ps.tile([DI, CH], f32, tag="mm")
                nc.tensor.matmul(ct_ps[:], W_C, XC[:, sl],
                                 start=True, stop=True)
                nc.vector.tensor_scalar(CT[:, sl], ct_ps[:], 0.0, None,
                                        op0=Alu.add)

            # ---- XCD = XC*Dp on Pool (runs during scan phase) ----
            XCD = bpool.tile([DI, L], bf16, name="XCD", tag="ESB")
            for hf in range(2):
                hsl = slice(hf * (L // 2), (hf + 1) * (L // 2))
                nc.gpsimd.tensor_tensor(XCD[:, hsl], XC[:, hsl],
                                        Dp.to_broadcast((DI, L // 2)),
                                        op=Alu.mult)

            # ---- scan phase: per channel-group g ----
            LH0 = L // 2
            ypsA = [psy.tile([DI, CH], f32, name=f"ypsA{ci}", tag="yps")
                    for ci in range(4)]
            for g in range(NG):
                udt = udpool.tile([DI, 2 * L], bf16, tag="udt")
                for j in range(NS):
                    nc.sync.dma_start(
                        udt[j * 16:(j + 1) * 16, :],
                        ud_dram[g * 16:(g + 1) * 16, :])
                for hf in range(2):
                    hsl = slice(hf * LH0, (hf + 1) * LH0)
                    dA = dapool.tile([DI, LH0], f32, tag="dA")
                    nc.scalar.activation(dA[:],
                                         udt[:, L + hf * LH0:L + (hf + 1) * LH0],
                                         Act.Exp, scale=a_vec[:, g:g + 1])
                    # dbx in-place into the U-half of udt (dead after this)
                    nc.vector.tensor_tensor(udt[:, hsl], udt[:, hsl],
                                            BT[:, hsl], op=Alu.mult)
                    init = 0.0 if hf == 0 else YPs[g][:, LH0 - 1:LH0]
                    nc.vector.tensor_tensor_scan(YPs[g][:, hsl], dA[:],
                                                 udt[:, hsl],
                                                 init, op0=Alu.mult, op1=Alu.add)
                # y partial: YP = H * C_tile (even groups on Pool)
                if g % 2 == 0:
                    nc.gpsimd.tensor_tensor(YPs[g][:], YPs[g][:], CT[:],
                                            op=Alu.mult)
                else:
                    nc.vector.tensor_tensor(YPs[g][:], YPs[g][:], CT[:],
                                            op=Alu.mult)
                # incremental y-reduce for chunks 0-3 (PSUM live across phase)
                for ci in range(4):
                    slc = slice(ci * CH, (ci + 1) * CH)
                    nc.tensor.matmul(ypsA[ci][:], Rg[:, g * 128:(g + 1) * 128],
                                     YPs[g][:, slc],
                                     start=(g == 0), stop=(g == NG - 1))

            # ---- y reduce: chunks 0-3 done incrementally; drain + chunks 4-7
            for chi in range(4):
                sl = slice(chi * CH, (chi + 1) * CH)
                ysb = spool.tile([DI, CH], bf16, tag="ysb")
                nc.scalar.activation(ysb[:], ypsA[chi][:], Act.Copy)
                nc.sync.dma_start(
                    y_in_t[chi // 2, :, (chi % 2) * CH:(chi % 2 + 1) * CH],
                    ysb[:])
            for chi in range(4, NCH):
                sl = slice(chi * CH, (chi + 1) * CH)
                yps = psy.tile([DI, CH], f32, tag="yps")
                for g in range(NG):
                    nc.tensor.matmul(yps[:], Rg[:, g * 128:(g + 1) * 128],
                                     YPs[g][:, sl],
                                     start=(g == 0), stop=(g == NG - 1))
                ysb = spool.tile([DI, CH], bf16, tag="ysb")
                if chi % 2 == 0:
                    nc.scalar.activation(ysb[:], yps[:], Act.Copy)
                else:
                    nc.vector.tensor_scalar(ysb[:], yps[:], 0.0, None,
                                            op0=Alu.add)
                nc.sync.dma_start(
                    y_in_t[chi // 2, :, (chi % 2) * CH:(chi % 2 + 1) * CH],
                    ysb[:])

            # ---- AllReduce partial y in quarters, pipelined with post/out ----
            # XCD precomputed on Pool (overlaps scan phase)
            YS = bpool.tile([DI, L], bf16, name="YS", tag="HN")
            LQ = L // 4
            for q in range(4):
                qsl = slice(q * LQ, (q + 1) * LQ)
                if sim:
                    nc.sync.dma_start(y_out_t[q], y_in_t[q])
                else:
                    nc.gpsimd.collective_compute(
                        "AllReduce", Alu.add, replica_groups=groups,
                        ins=[y_in_t[q].opt()], outs=[y_out_t[q].opt()])
                nc.sync.dma_start(YSUM[:, qsl], y_out_t[q])
                nc.vector.tensor_add(XCD[:, qsl], YSUM[:, qsl], XCD[:, qsl])
                nc.vector.tensor_mul(YS[:, qsl], XCD[:, qsl], ZS[:, qsl])
                for ci in range(2):
                    chi = q * 2 + ci
                    sl = slice(chi * CH, (chi + 1) * CH)
                    op_ps = psy.tile([C, CH], f32, tag="yps")
                    nc.tensor.matmul(op_ps[:], op_lhsT, YS[:, sl],
                                     start=True, stop=False)
                    nc.tensor.matmul(op_ps[:], ident[0:C, 0:C], SEQ[:, sl],
                                     start=False, stop=True)
                    outc = spool.tile([C, CH], f32, tag="outc")
                    if ci == 0:
                        nc.vector.tensor_scalar(outc[:], op_ps[:], 0.0, None,
                                                op0=Alu.add)
                    else:
                        nc.scalar.activation(outc[:], op_ps[:], Act.Copy)
                    nc.sync.dma_start(out_d[:, sl], outc[:])

    nc.compile()
    return nc


def _host_precompute(inp):
    import ml_dtypes
    f = lambda k: np.asarray(inp[k], np.float32)
    bf = lambda a: np.ascontiguousarray(a.astype(ml_dtypes.bfloat16))
    w1 = f("conv_w")[:, :, 0, 0]
    wh = f("dwh_w")[:, 0, :, 0]
    ww = f("dww_w")[:, 0, 0, :]
    taps = [
        w1 * (1.0 + wh[:, 1] + ww[:, 1])[None, :],   # center
        w1 * wh[:, 0][None, :],                       # up
        w1 * wh[:, 2][None, :],                       # down
        w1 * ww[:, 0][None, :],                       # left
        w1 * ww[:, 2][None, :],                       # right
    ]
    cw = np.concatenate([t.T for t in taps], axis=1)  # [cin=64, 5*64]
    btot = f("conv_b") + w1 @ (f("dwh_b") + f("dww_b"))
    s_bn = f("bn_g") / np.sqrt(f("bn_v") + EPS)
    bn_bias = s_bn * (btot - f("bn_m")) + f("bn_b")
    ipw = f("in_proj_w")
    ip_lhsT = (ipw * f("ln_g")[None, :]).T            # [64, 256]
    ip_bias = ipw @ f("ln_b")                          # [256]
    xpw = f("x_proj_w")                                # [36, 128]
    M_dt = f("dt_proj_w") @ xpw[:DR]                   # [128, 128]
    a_full = -np.exp(np.asarray(inp["A_log"], np.float32))  # [DI, DS]
    cdw = f("convd_w")[:, 0, :]                        # [128, 4]

    per_sigma = []
    for sg in range(2):
        s_lo = sg * NS
        cf32 = np.zeros((128, 32), np.float32)
        cf32[:C, 0] = s_bn
        cf32[:C, 1] = bn_bias
        cf32[:, 2] = ip_bias[:DI]
        cf32[:, 3] = ip_bias[DI:]
        cf32[:, 4] = f("convd_b")
        cf32[:, 5] = f("dt_proj_b")
        cf32[:, 6] = f("Dp")
        # a_vec per group g: a[p] = a_full[16g + p%16, s_lo + p//16]
        p = np.arange(128)
        for g in range(NG):
            cf32[:, 8 + g] = a_full[16 * g + p % 16, s_lo + p // 16]

        cbf = np.zeros((128, 2688), np.float32)
        cbf[:, 0:128] = np.eye(128, dtype=np.float32)
        cbf[:C, 128:448] = cw
        cbf[:C, 448:704] = ip_lhsT
        cbf[:, 704:832] = M_dt.T
        for tap in range(4):
            cbf[:, 832 + tap * 128:832 + (tap + 1) * 128] = np.diag(cdw[:, tap])
        # fused B/C broadcast: W_B[p, :] = xpw_B[s_lo + p//16, :] (stored T)
        for pp in range(128):
            cbf[:, 1344 + pp] = xpw[DR + s_lo + pp // 16]
            cbf[:, 1472 + pp] = xpw[DR + DS + s_lo + pp // 16]
        # Rg: R_g[p, d] = 1 iff d == 16g + p%16
        for g in range(NG):
            for pp in range(128):
                cbf[pp, 1600 + g * 128 + 16 * g + pp % 16] = 1.0
        cbf[:, 2624:2688] = f("out_proj_w").T
        per_sigma.append(dict(cf32=cf32, cbf=bf(cbf)))
    return {}, per_sigma


def _shift_images(xb):
    # 5 pre-shifted copies: ctr, up(reads h-1), dn(h+1), lf(w-1), rt(w+1)
    import ml_dtypes
    out = np.zeros((C, 5, H, W), np.float32)
    out[:, 0] = xb
    out[:, 1, 1:, :] = xb[:, :-1, :]
    out[:, 2, :-1, :] = xb[:, 1:, :]
    out[:, 3, :, 1:] = xb[:, :, :-1]
    out[:, 4, :, :-1] = xb[:, :, 1:]
    return np.ascontiguousarray(
        out.transpose(1, 0, 2, 3).reshape(5, C, L).transpose(1, 0, 2)
        .reshape(C, 5 * L).astype(ml_dtypes.bfloat16))


TRACE = False
LAST_EXEC_NS = None
LAST_TRACE_DIR = None


def kernel(**inputs):
    global LAST_EXEC_NS, LAST_TRACE_DIR
    from concourse.bass_utils import run_bass_kernel_spmd

    if "nc" not in _cached:
        _cached["nc"] = _build_program()
    nc = _cached["nc"]

    common, per_sigma = _host_precompute(inputs)
    x = np.asarray(inputs["x"], np.float32)
    in_maps = []
    for c in range(NCORES):
        b, sg = c // 2, c % 2
        m = dict(common)
        m.update(per_sigma[sg])
        m["ximgs"] = _shift_images(x[b])
        in_maps.append(m)

    kw = {}
    if TRACE:
        import tempfile
        LAST_TRACE_DIR = tempfile.mkdtemp(prefix="bass_trace_")
        kw = dict(trace=True, tmpdir=LAST_TRACE_DIR)
    r = run_bass_kernel_spmd(nc, in_maps, list(range(NCORES)), **kw)
    if r.exec_time_ns is not None:
        LAST_EXEC_NS = r.exec_time_ns
    res = r.results
    out = np.empty((B, C, H, W), np.float32)
    for b in range(B):
        out[b] = np.asarray(res[2 * b]["out_f"], np.float32).reshape(C, H, W)
    return out
